# revision 53
# baseline (speedup 1.0000x reference)
"""Trainium2 Bass kernel for nn_MicroBiMambaBackbone (v3).

Data-parallel over batch (B=8 -> 8 cores, 1 sample/core).
Measured (clean clock state): 533us, rel err 4.1e-4. Each unit's post ops
defer into the NEXT unit's scan (s==1 callback) so dA0/dBx0 aren't queued
behind them. Measured-optimal knobs (do not retune blindly): MID_STATES
(3,6,9,12) — (2,4,7,10) cost ~5us; mean-pool as reduce(h_out) — splitting
into reduce(h_in)+reduce(o_s) cost ~2us of extra DVE work. Vector ~99%
occupied = the wall: 282us scan (tensor_tensor_scan is 0.5 elem/cy/lane,
inherent) + 174us dBx/pmult tensor_tensor with no legal alternate engine.
Remaining vector idle is ~92us: ~16us fixed tail (NOTIFY/teardown), ~50us
layer-0 front latency (serial LN->conv->dt->delta chain, nothing for DVE),
rest mid-scan Act/DMA waits.
Changes vs v2 (~580us -> ~539us; vector engine now ~99% occupied):
  - Embedding inputs + early-front weights DMA'd before the ~1MB of late
    weights; dA/dBx triple-buffered so Act runs 2 states ahead of the scan.
  - Scan partition layout (d8,n16) -> (d32,n4): delta/u replicate x4 instead
    of x16, B/C broadcast from compact (16,L) DRAM scratch instead of
    PE-tiled production. DMA traffic ~17MB -> ~9MB per layer (DMA engine
    union-active 475us -> 156us), which removed DMA-gated scan stalls.
  - n-reduction via sum4 stationaries (two 64-col parity variants since
    PSUM matmul base partition must be 0/32/64).
  - Full-length (FD=2048) tensor_tensor_scan per state tile (was 2 halves).
  - DMA issue order: dl0/ur0 before the bm/cm prefetch, dl/ur one b-block
    ahead; scratch written in 32-row blocks so b=0 broadcast starts early.
  - Mean-pool reduces issue as soon as each chain finishes, into zv halves.
Measured dead ends (do not retry): gpsimd tensor_tensor (6.2us/op AND
inflates concurrent DVE tensor_tensor ~30% via the shared SBUF port);
DMA CCE multiply (NCC_IBIR077: only add-family supported in Copy mode);
chunked PE-scan (dBx cannot be produced transposed: it is elementwise in t);
exp/ln table fusion (compiler first-match puts Exp and Ln in different sets).
Run-to-run HW clock variance is ~+/-20% (observed 542..647us for identical
code); compare traces by instruction counts/avgs, not wall time alone.
"""

import sys

sys.path.insert(0, "/opt/trn_rl_repo")

from contextlib import ExitStack

import ml_dtypes
import numpy as np

import concourse.bacc as bacc
import concourse.bass as bass
import concourse.mybir as mybir
import concourse.tile as tile

BF = mybir.dt.bfloat16
F32 = mybir.dt.float32
AF = mybir.ActivationFunctionType
OP = mybir.AluOpType

B, L_FULL, IN_DIM = 8, 2048, 5
D_MODEL, OUT_DIM = 64, 64
N_LAYERS, D_INNER, N_STATE, DT_RANK, K = 2, 128, 16, 4, 4
T = 2 * N_LAYERS
N_CORES = 8

MM_F = 512  # max matmul free dim (one PSUM bank of f32)

# gpsimd tensor_tensor confirmed poisonous: 6.2us/op and inflates concurrent
# DVE tensor_tensor by ~30% (shared SBUF port). Keep everything off gpsimd.
MID_STATES = (3, 6, 9, 12)  # scan states after which front phases issue


def _mm(nc, out, lhsT, rhs, start=True, stop=True):
    F = rhs.shape[-1]
    for j in range(0, F, MM_F):
        e = min(j + MM_F, F)
        nc.tensor.matmul(out[:, j:e], lhsT, rhs[:, j:e], start=start, stop=stop)


def build_nc(L=L_FULL):
    nc = bacc.Bacc("TRN2", target_bir_lowering=False)
    H = L // 2

    # ---------------- DRAM I/O ----------------
    d_xT = nc.dram_tensor("xT", (IN_DIM, L), BF, kind="ExternalInput")
    d_Wemb = nc.dram_tensor("Wemb", (IN_DIM, D_MODEL), BF, kind="ExternalInput")
    d_bemb = nc.dram_tensor("bemb", (D_MODEL, 1), F32, kind="ExternalInput")
    d_peT = nc.dram_tensor("peT", (D_MODEL, L), BF, kind="ExternalInput")
    d_Wstat = nc.dram_tensor("Wstat", (D_MODEL, 2 * D_MODEL), BF, kind="ExternalInput")
    d_Win = nc.dram_tensor("Win", (D_MODEL, T * 2 * D_INNER), BF, kind="ExternalInput")
    d_beta = nc.dram_tensor("beta", (D_INNER, 2 * T), F32, kind="ExternalInput")
    d_cdiag = nc.dram_tensor("cdiag", (D_INNER, T * K * D_INNER), BF, kind="ExternalInput")
    d_bconv = nc.dram_tensor("bconv", (D_INNER, T), F32, kind="ExternalInput")
    d_Wxdt = nc.dram_tensor("Wxdt", (D_INNER, T * DT_RANK), BF, kind="ExternalInput")
    d_Wdt = nc.dram_tensor("Wdt", (DT_RANK, T * D_INNER), BF, kind="ExternalInput")
    d_bdt = nc.dram_tensor("bdt", (D_INNER, T), F32, kind="ExternalInput")
    d_WxB = nc.dram_tensor("WxB", (D_INNER, T * N_STATE), BF, kind="ExternalInput")
    d_WxC = nc.dram_tensor("WxC", (D_INNER, T * N_STATE), BF, kind="ExternalInput")
    d_sum4 = nc.dram_tensor("sum4", (D_INNER, 2 * 64), BF, kind="ExternalInput")
    d_dskd = nc.dram_tensor("dskd", (D_INNER, T * D_INNER), BF, kind="ExternalInput")
    d_Acol = nc.dram_tensor("Acol", (D_INNER, T * N_STATE), F32, kind="ExternalInput")
    d_Wout = nc.dram_tensor("Wout", (D_INNER, T * D_MODEL), BF, kind="ExternalInput")
    d_Wproj = nc.dram_tensor("Wproj", (2 * D_MODEL, OUT_DIM), F32, kind="ExternalInput")
    d_bproj = nc.dram_tensor("bproj", (OUT_DIM, 1), F32, kind="ExternalInput")
    d_out = nc.dram_tensor("out", (OUT_DIM, 1), F32, kind="ExternalOutput")

    with ExitStack() as ctx:
        tc = ctx.enter_context(tile.TileContext(nc))
        wp = ctx.enter_context(tc.tile_pool(name="weights", bufs=1))
        hp = ctx.enter_context(tc.tile_pool(name="hres", bufs=2))
        ap = ctx.enter_context(tc.tile_pool(name="acts", bufs=1))
        sp = ctx.enter_context(tc.tile_pool(name="scan", bufs=2))
        spr = ctx.enter_context(tc.tile_pool(name="rep", bufs=2))
        bcp = ctx.enter_context(tc.tile_pool(name="bc", bufs=1))
        dbxp = ctx.enter_context(tc.tile_pool(name="dbx", bufs=1))
        dp = ctx.enter_context(tc.tile_pool(name="dscr", bufs=1, space="DRAM"))
        psf = ctx.enter_context(tc.tile_pool(name="psF", bufs=1, space="PSUM"))
        psy = ctx.enter_context(tc.tile_pool(name="psY", bufs=1, space="PSUM"))

        # ---------------- load weights ----------------
        def wload(d, shape, dtype, nsplit=1):
            t = wp.tile(list(shape), dtype, tag="w_" + d.name)
            f = shape[1]
            step = (f + nsplit - 1) // nsplit
            for j in range(0, f, step):
                e = min(j + step, f)
                nc.sync.dma_start(t[:, j:e], d[:, j:e])
            return t

        s_Wemb = wload(d_Wemb, (IN_DIM, D_MODEL), BF)
        s_bemb = wload(d_bemb, (D_MODEL, 1), F32)

        # ---------------- embedding (inputs DMA'd before the heavy weights —
        # Win/cdiag are ~1MB — so layer-0's front isn't queued behind them) ---
        with tc.tile_pool(name="embin", bufs=1) as ep:
            s_xT = ep.tile([IN_DIM, L], BF, tag="xT")
            nc.sync.dma_start(s_xT[:], d_xT[:])
            s_peT = ep.tile([D_MODEL, L], BF, tag="peT")
            for j in (0, H):
                nc.sync.dma_start(s_peT[:, j:j + H], d_peT[:, j:j + H])
            s_Wstat = wload(d_Wstat, (D_MODEL, 2 * D_MODEL), BF)
            s_Win = wload(d_Win, (D_MODEL, T * 2 * D_INNER), BF)
            s_beta = wload(d_beta, (D_INNER, 2 * T), F32)
            s_cdiag = wload(d_cdiag, (D_INNER, T * K * D_INNER), BF, nsplit=2)
            s_bconv = wload(d_bconv, (D_INNER, T), F32)
            s_Wxdt = wload(d_Wxdt, (D_INNER, T * DT_RANK), BF)
            s_Wdt = wload(d_Wdt, (DT_RANK, T * D_INNER), BF)
            s_bdt = wload(d_bdt, (D_INNER, T), F32)
            s_WxB = wload(d_WxB, (D_INNER, T * N_STATE), BF)
            s_WxC = wload(d_WxC, (D_INNER, T * N_STATE), BF)
            s_eps = wp.tile([D_MODEL, 1], F32, tag="eps")
            nc.vector.memset(s_eps[:], 1e-5)
            s_Acol = wload(d_Acol, (D_INNER, T * N_STATE), F32)
            s_sum4 = wload(d_sum4, (D_INNER, 2 * 64), BF)
            s_dskd = wload(d_dskd, (D_INNER, T * D_INNER), BF)
            s_Wout = wload(d_Wout, (D_INNER, T * D_MODEL), BF)
            s_Wproj = wload(d_Wproj, (2 * D_MODEL, OUT_DIM), F32)
            s_bproj = wload(d_bproj, (OUT_DIM, 1), F32)
            h_f = hp.tile([D_MODEL, L], BF, tag="hf")
            for j in (0, H):
                eP = psf.tile([D_INNER, H], F32, tag="mmA")
                _mm(nc, eP[0:D_MODEL, :], s_Wemb[:], s_xT[:, j:j + H])
                nc.vector.scalar_tensor_tensor(
                    h_f[:, j:j + H], eP[0:D_MODEL, :], s_bemb[:], s_peT[:, j:j + H],
                    OP.add, OP.add)
            h_b = hp.tile([D_MODEL, L], BF, tag="hb")
            nc.scalar.activation(h_b[:], h_f[:, ::-1], AF.Identity)

        # ---------------- one mamba layer ----------------
        def front_phases(l, c, h_in):
            """LN + in-proj + conv + dt + u + bm/cm + scratch writes."""
            t = {}
            # --- LN: hm = (I - J/64) h ; var = J/64 hm^2 ---
            hmb = ap.tile([D_MODEL, L], BF, tag="hmb")
            inv = ap.tile([D_MODEL, L], BF, tag="inv")
            hmP = [psf.tile([D_INNER, H], F32, tag=tg, name="hmP" + tg) for tg in ("mmA", "mmB")]
            sq = [ap.tile([D_MODEL, H], BF, tag=tg, name="t" + tg) for tg in ("sq0", "sq1")]
            for i, j in enumerate((0, H)):
                _mm(nc, hmP[i][0:D_MODEL, :], s_Wstat[:, 0:D_MODEL], h_in[:, j:j + H])
            for i in (0, 1):
                nc.scalar.activation(sq[i][:], hmP[i][0:D_MODEL, :], AF.Square)
            for i, j in enumerate((0, H)):
                nc.scalar.activation(hmb[:, j:j + H], hmP[i][0:D_MODEL, :], AF.Identity)
            varP = [psf.tile([D_INNER, H], F32, tag=tg, name="varP" + tg) for tg in ("mmA", "mmB")]
            for i in (0, 1):
                _mm(nc, varP[i][0:D_MODEL, :], s_Wstat[:, D_MODEL:2 * D_MODEL], sq[i][:])
            for i, j in enumerate((0, H)):
                nc.scalar.activation(inv[:, j:j + H], varP[i][0:D_MODEL, :],
                                     AF.Abs_reciprocal_sqrt, bias=s_eps[:])
            hn = ap.tile([D_MODEL, L], BF, tag="hn")
            nc.vector.tensor_tensor(hn[:], hmb[:], inv[:], OP.mult)
            yield t

            # --- in-proj (xi into padded conv input, z -> silu) ---
            xi = ap.tile([D_INNER, L + K - 1], BF, tag="xi")
            nc.vector.memset(xi[:, 0:K - 1], 0.0)
            sz = ap.tile([D_INNER, L], BF, tag="sz" + c)
            w_in = s_Win[:, l * 2 * D_INNER:(l + 1) * 2 * D_INNER]
            for j in (0, H):
                xiP = psf.tile([D_INNER, H], F32, tag="mmA")
                _mm(nc, xiP, w_in[:, 0:D_INNER], hn[:, j:j + H])
                nc.scalar.activation(xi[:, K - 1 + j:K - 1 + j + H], xiP[:],
                                     AF.Identity, bias=s_beta[:, 2 * l:2 * l + 1])
                zP = psf.tile([D_INNER, H], F32, tag="mmB")
                _mm(nc, zP, w_in[:, D_INNER:2 * D_INNER], hn[:, j:j + H])
                nc.scalar.activation(sz[:, j:j + H], zP[:], AF.Silu,
                                     bias=s_beta[:, 2 * l + 1:2 * l + 2])
            # --- causal depthwise conv (4 diag matmuls) + silu ---
            xc = ap.tile([D_INNER, L], BF, tag="xc" + c)
            cP2 = [psf.tile([D_INNER, H], F32, tag=tg, name="cP" + tg) for tg in ("mmA", "mmB")]
            for k in range(K):  # k outer: stationary conv weight reused across halves
                dg = s_cdiag[:, (l * K + k) * D_INNER:(l * K + k + 1) * D_INNER]
                for i, j in enumerate((0, H)):
                    _mm(nc, cP2[i], dg, xi[:, j + k:j + k + H],
                        start=(k == 0), stop=(k == K - 1))
            for i, j in enumerate((0, H)):
                nc.scalar.activation(xc[:, j:j + H], cP2[i][:], AF.Silu,
                                     bias=s_bconv[:, l:l + 1])
            yield t
            # --- dt path -> delta = softplus(dt @ Wdt + b_dt) ---
            dt_bf = ap.tile([DT_RANK, L], BF, tag="dtbf")
            for j in (0, H):
                dtP = psf.tile([D_INNER, H], F32, tag="mmB")
                _mm(nc, dtP[0:DT_RANK, :], s_Wxdt[:, l * DT_RANK:(l + 1) * DT_RANK],
                    xc[:, j:j + H])
                nc.scalar.activation(dt_bf[:, j:j + H], dtP[0:DT_RANK, :], AF.Identity)
            delta = ap.tile([D_INNER, L], BF, tag="delta")
            dpP = [psf.tile([D_INNER, H], F32, tag=tg, name="dpP" + tg) for tg in ("mmA", "mmB")]
            dexp = [ap.tile([D_INNER, H], BF, tag=tg, name="t" + tg) for tg in ("dexp0", "dexp1")]
            for i, j in enumerate((0, H)):
                _mm(nc, dpP[i], s_Wdt[:, l * D_INNER:(l + 1) * D_INNER],
                    dt_bf[:, j:j + H])
            for i in (0, 1):
                nc.scalar.activation(dexp[i][:], dpP[i][:], AF.Exp,
                                     bias=s_bdt[:, l:l + 1])
            for i, j in enumerate((0, H)):
                nc.scalar.activation(delta[:, j:j + H], dexp[i][:], AF.Ln, bias=1.0)
            # --- u = delta * xc ---
            u = ap.tile([D_INNER, L], BF, tag="u")
            nc.vector.tensor_tensor(u[:], delta[:], xc[:], OP.mult)
            yield t
            # --- B/C compact rows (B at partition 0, C at 32: engine partition
            # base must be 0/32/64/96) + scratch DRAM write for replication ---
            sBC = ap.tile([32 + N_STATE, L], BF, tag="sbc" + c)
            for r0, w_all in ((0, s_WxB), (32, s_WxC)):
                for j in (0, H):
                    rP = psf.tile([D_INNER, H], F32, tag="mmB")
                    _mm(nc, rP[0:N_STATE, :],
                        w_all[:, l * N_STATE:(l + 1) * N_STATE], xc[:, j:j + H])
                    nc.scalar.activation(sBC[r0:r0 + N_STATE, j:j + H],
                                         rP[0:N_STATE, :], AF.Identity)
            scrBC = dp.tile([32 + N_STATE, L], BF, tag="scrBC" + c)
            for j in (0, H):
                nc.sync.dma_start(scrBC[:, j:j + H], sBC[:, j:j + H])
            # --- scratch DRAM writes for delta/u replication (32-row blocks so
            # the b=0 broadcast can start after the first quarter lands) ---
            scrD = dp.tile([D_INNER, L], BF, tag="scrD" + c)
            scrU = dp.tile([D_INNER, L], BF, tag="scrU" + c)
            for r in range(0, D_INNER, 32):
                nc.sync.dma_start(scrD[r:r + 32, :], delta[r:r + 32, :])
                nc.sync.dma_start(scrU[r:r + 32, :], u[r:r + 32, :])
            t.update(xc=xc, sz=sz, scrD=scrD, scrU=scrU, scrBC=scrBC,
                     h_in=h_in, l=l, c=c)
            yield t

        def scan_phase(t, mid_cb=None, post_cb=None):
            l, c = t["l"], t["c"]
            yaccP = psy.tile([D_INNER, L], F32, tag="yacc")
            t["yaccP"] = yaccP
            # D_skip * xc seeds the psum accumulation (diag matmuls per block)
            for blk in (0, 64):
                dgw = s_dskd[:, l * D_INNER + blk:l * D_INNER + blk + 64]
                for j in range(0, L, MM_F):
                    nc.tensor.matmul(yaccP[blk:blk + 64, j:j + MM_F], dgw,
                                     t["xc"][:, j:j + MM_F], start=True, stop=False,
                                     skip_group_check=True)
            # DMA issue order matters (one HW queue): state 0 needs dl0/ur0/bm0/
            # cm0 first; later coefficient tiles and the next b-block prefetch
            # behind them.
            def issue_dlur(b):
                dl = spr.tile([D_INNER, L], BF, tag="dl")
                nc.sync.dma_start(
                    dl[:], t["scrD"][32 * b:32 * b + 32, :].unsqueeze(1)
                    .broadcast_to((32, 4, L)))
                ur = spr.tile([D_INNER, L], BF, tag="ur")
                nc.sync.dma_start(
                    ur[:], t["scrU"][32 * b:32 * b + 32, :].unsqueeze(1)
                    .broadcast_to((32, 4, L)))
                return dl, ur

            cur = issue_dlur(0)
            bmall = bcp.tile([D_INNER, 4, L], BF, tag="bmall")
            cmt = []
            for nq in range(4):
                nc.sync.dma_start(
                    bmall[:, nq, :], t["scrBC"][4 * nq:4 * nq + 4, :].unsqueeze(0)
                    .broadcast_to((32, 4, L)))
                cq = bcp.tile([D_INNER, L], BF, tag=f"cm{nq}")
                nc.sync.dma_start(
                    cq[:], t["scrBC"][32 + 4 * nq:32 + 4 * nq + 4, :]
                    .unsqueeze(0).broadcast_to((32, 4, L)))
                cmt.append(cq)
            s = 0
            for b in range(4):
                dl, ur = cur
                if b < 3:
                    cur = issue_dlur(b + 1)
                # all 4 states' dBx in one wide tensor_tensor (ur broadcast
                # along the nq axis): 3 fewer DVE instrs + sem waits per block
                dBxall = dbxp.tile([D_INNER, 4, L], BF, tag="dBx")
                nc.vector.tensor_tensor(
                    dBxall[:], ur[:].unsqueeze(1).broadcast_to((D_INNER, 4, L)),
                    bmall[:], OP.mult)
                for nq in range(4):
                    dA = sp.tile([D_INNER, L], BF, tag="dA")
                    nc.scalar.activation(
                        dA[:], dl[:], AF.Exp,
                        scale=s_Acol[:, l * N_STATE + 4 * b + nq:
                                     l * N_STATE + 4 * b + nq + 1])
                    hs = sp.tile([D_INNER, L], BF, tag="hs")
                    nc.vector.tensor_tensor_scan(hs[:], dA[:], dBxall[:, nq, :],
                                                 0.0, OP.mult, OP.add)
                    p = sp.tile([D_INNER, L], BF, tag="p")
                    nc.vector.tensor_tensor(p[:], cmt[nq][:], hs[:], OP.mult)
                    # PSUM base partition must be 0/32/64: write a 64-row block
                    # (base (b//2)*64); the parity-variant stationary routes this
                    # b's 32 rows, the other 32 columns are zero (accumulate +0).
                    base, par = (b // 2) * 64, b % 2
                    for j in range(0, L, MM_F):
                        nc.tensor.matmul(yaccP[base:base + 64, j:j + MM_F],
                                         s_sum4[:, par * 64:par * 64 + 64],
                                         p[:, j:j + MM_F],
                                         start=False, stop=(b % 2 == 1 and nq == 3),
                                         skip_group_check=True)
                    # previous unit's post is deferred to here (s==1) so this
                    # unit's dA0/dBx0/scan0 aren't queued behind its copies
                    if post_cb is not None and s == 1:
                        post_cb()
                    if mid_cb is not None and s in MID_STATES:
                        mid_cb()
                    s += 1

        def post_part(t):
            # --- postprocess: y = yacc (has D_skip term), gate, out-proj ---
            l, c = t["l"], t["c"]
            yaccP = t["yaccP"]
            yaccS = ap.tile([D_INNER, L], BF, tag="yac" + c)
            for j in (0, H):
                nc.scalar.activation(yaccS[:, j:j + H], yaccP[:, j:j + H], AF.Identity)
            yg = ap.tile([D_INNER, L], BF, tag="yg" + c)
            nc.vector.tensor_tensor(yg[:], yaccS[:], t["sz"], OP.mult)
            o_s = ap.tile([D_MODEL, L], BF, tag="os" + c)
            for j in (0, H):
                oP = psf.tile([D_INNER, H], F32, tag="mmA")
                _mm(nc, oP[0:D_MODEL, :], s_Wout[:, l * D_MODEL:(l + 1) * D_MODEL],
                    yg[:, j:j + H])
                nc.scalar.activation(o_s[:, j:j + H], oP[0:D_MODEL, :], AF.Identity)
            h_out = hp.tile([D_MODEL, L], BF, tag="h" + c)
            nc.vector.tensor_tensor(h_out[:], t["h_in"][:], o_s[:], OP.add)
            return h_out

        # ---------------- run the 4 layer-units, fronts interleaved ----------
        units = [(0, "f"), (N_LAYERS, "b"), (1, "f"), (N_LAYERS + 1, "b")]
        hcur = {"f": h_f, "b": h_b}

        def run_all(gen):
            t = None
            for t in gen:
                pass
            return t

        zv = ap.tile([2 * D_MODEL, 1], F32, tag="zv")
        ctx_next = run_all(front_phases(*units[0], hcur[units[0][1]]))
        prev_t = None
        for i in range(len(units)):
            t_cur = ctx_next
            holder = {}

            def mk_cb(idx, holder):
                if idx + 1 >= len(units):
                    return None
                ln, cn = units[idx + 1]
                state = {}

                def cb():
                    # generator built lazily: hcur[cn] is updated by the
                    # deferred post (s==1) before the first front phase (s==3)
                    if "gen" not in state:
                        state["gen"] = front_phases(ln, cn, hcur[cn])
                    try:
                        holder["ctx"] = next(state["gen"])
                    except StopIteration:
                        pass
                return cb

            def mk_post(pt, pidx):
                if pt is None:
                    return None

                def cb():
                    h_new = post_part(pt)
                    hcur[pt["c"]] = h_new
                    if pidx == 2:  # f-chain done: mean-pool it now
                        nc.vector.tensor_reduce(
                            zv[0:D_MODEL, :], h_new[:],
                            axis=mybir.AxisListType.X, op=OP.add)
                return cb

            scan_phase(t_cur, mk_cb(i, holder), mk_post(prev_t, i - 1))
            prev_t = t_cur
            ctx_next = holder.get("ctx")
        h_last = post_part(prev_t)
        hcur[prev_t["c"]] = h_last
        nc.vector.tensor_reduce(zv[D_MODEL:2 * D_MODEL, :], h_last[:],
                                axis=mybir.AxisListType.X, op=OP.add)

        # ---------------- head ----------------
        oP = psf.tile([D_INNER, 1], F32, tag="mmB")
        nc.tensor.matmul(oP[0:OUT_DIM, :], s_Wproj[:], zv[:])
        ofin = ap.tile([OUT_DIM, 1], F32, tag="ofin")
        nc.scalar.activation(ofin[:], oP[0:OUT_DIM, :], AF.Identity,
                             bias=s_bproj[:])
        nc.sync.dma_start(d_out[:], ofin[:])

    return nc


def prep_inputs(inputs, L=L_FULL):
    bf = ml_dtypes.bfloat16
    f32 = np.float32
    g = {k: np.asarray(v) for k, v in inputs.items()}
    W_in, W_conv, W_x, W_dt = g["W_in"], g["W_conv"], g["W_x"], g["W_dt"]
    ln_w, ln_b = g["ln_w"], g["ln_b"]

    Win = np.concatenate([W_in[l] * ln_w[l][:, None] for l in range(T)], axis=1)
    beta = np.stack([ln_b[l] @ W_in[l] for l in range(T)], 0)
    beta_blob = np.zeros((D_INNER, 2 * T), f32)
    for l in range(T):
        beta_blob[:, 2 * l] = beta[l, :D_INNER]
        beta_blob[:, 2 * l + 1] = beta[l, D_INNER:]
    cdiag = np.zeros((D_INNER, T * K * D_INNER), f32)
    for l in range(T):
        for k in range(K):
            blk = (l * K + k) * D_INNER
            cdiag[np.arange(D_INNER), blk + np.arange(D_INNER)] = W_conv[l, :, 0, k]
    Wxdt = np.concatenate([W_x[l][:, :DT_RANK] for l in range(T)], axis=1)
    Wdt = np.concatenate([W_dt[l] for l in range(T)], axis=1)
    WxB = np.concatenate(
        [W_x[l][:, DT_RANK:DT_RANK + N_STATE] for l in range(T)], axis=1)
    WxC = np.concatenate(
        [W_x[l][:, DT_RANK + N_STATE:] for l in range(T)], axis=1)
    # (d32, n4) partition layout: p = d32*4 + j; sum over the 4 j-partitions.
    # Two 64-col variants: even b -> rows land in cols 0-31, odd b -> cols 32-63.
    sum4 = np.zeros((D_INNER, 2 * 64), f32)
    for i in range(32):
        for j in range(4):
            sum4[4 * i + j, i] = 1.0
            sum4[4 * i + j, 64 + 32 + i] = 1.0
    # diag(D_skip) blocks for the yacc seed matmuls
    dskd = np.zeros((D_INNER, T * D_INNER), f32)
    for l in range(T):
        for blk in (0, 64):
            for q in range(64):
                dskd[blk + q, l * D_INNER + blk + q] = g["D_skip"][l][blk + q]
    A = -np.exp(g["A_log"])
    Acol = np.zeros((D_INNER, T * N_STATE), f32)
    pidx = np.arange(D_INNER)
    for l in range(T):
        for b in range(4):
            for nq in range(4):
                Acol[:, l * N_STATE + 4 * b + nq] = A[l][32 * b + pidx // 4,
                                                         4 * nq + pidx % 4]
    Wout = np.concatenate([g["W_out"][l] for l in range(T)], axis=1)
    # LN stats weights: [I - J/64 | J/64]
    Wstat = np.zeros((D_MODEL, 2 * D_MODEL), f32)
    Wstat[:, 0:D_MODEL] = np.eye(D_MODEL) - 1.0 / D_MODEL
    Wstat[:, D_MODEL:] = 1.0 / D_MODEL

    shared = {
        "Wemb": g["W_emb"].astype(bf),
        "bemb": g["b_emb"].reshape(D_MODEL, 1).astype(f32),
        "peT": np.ascontiguousarray(g["pe"][:L].T).astype(bf),
        "Wstat": Wstat.astype(bf),
        "Win": Win.astype(bf),
        "beta": beta_blob,
        "cdiag": cdiag.astype(bf),
        "bconv": np.ascontiguousarray(g["b_conv"].T).astype(f32),
        "Wxdt": Wxdt.astype(bf),
        "Wdt": Wdt.astype(bf),
        "bdt": np.ascontiguousarray(g["b_dt"].T).astype(f32),
        "WxB": WxB.astype(bf),
        "WxC": WxC.astype(bf),
        "sum4": sum4.astype(bf),
        "dskd": dskd.astype(bf),
        "Acol": Acol.astype(f32),
        "Wout": Wout.astype(bf),
        "Wproj": (g["W_proj"] / L).astype(f32),
        "bproj": g["b_proj"].reshape(OUT_DIM, 1).astype(f32),
    }
    in_maps = []
    for c in range(B):
        m = dict(shared)
        m["xT"] = np.ascontiguousarray(g["x"][c, :L].T).astype(bf)
        in_maps.append(m)
    return in_maps


_CACHE = {}


def kernel(**inputs):
    if "nc" not in _CACHE:
        _CACHE["nc"] = build_nc()
        _CACHE["nc"].finalize()
    nc = _CACHE["nc"]
    in_maps = prep_inputs(inputs)
    from concourse.bass_utils import run_bass_kernel_spmd
    res = run_bass_kernel_spmd(nc, in_maps, core_ids=list(range(N_CORES)))
    out = np.stack([np.asarray(res.results[c]["out"]).reshape(OUT_DIM)
                    for c in range(N_CORES)], axis=0)
    return out.astype(np.float32)



# revision 57
# speedup vs baseline: 1.0460x; 1.0460x over previous
"""Trainium2 Bass kernel for nn_MicroBiMambaBackbone (v3).

Data-parallel over batch (B=8 -> 8 cores, 1 sample/core).
Measured (clean clock state): 539us, rel err 4.1e-4. Vector engine ~99%
occupied = the wall: 282us scan (tensor_tensor_scan is 0.5 elem/cy/lane,
inherent) + 174us dBx/pmult tensor_tensor with no legal alternate engine.
Remaining vector idle is ~92us: ~16us fixed tail (NOTIFY/teardown), ~50us
layer-0 front latency (serial LN->conv->dt->delta chain, nothing for DVE),
rest mid-scan Act/DMA waits.
Changes vs v2 (~580us -> ~539us; vector engine now ~99% occupied):
  - Embedding inputs + early-front weights DMA'd before the ~1MB of late
    weights; dA/dBx triple-buffered so Act runs 2 states ahead of the scan.
  - Scan partition layout (d8,n16) -> (d32,n4): delta/u replicate x4 instead
    of x16, B/C broadcast from compact (16,L) DRAM scratch instead of
    PE-tiled production. DMA traffic ~17MB -> ~9MB per layer (DMA engine
    union-active 475us -> 156us), which removed DMA-gated scan stalls.
  - n-reduction via sum4 stationaries (two 64-col parity variants since
    PSUM matmul base partition must be 0/32/64).
  - Full-length (FD=2048) tensor_tensor_scan per state tile (was 2 halves).
  - DMA issue order: dl0/ur0 before the bm/cm prefetch, dl/ur one b-block
    ahead; scratch written in 32-row blocks so b=0 broadcast starts early.
  - Mean-pool reduces issue as soon as each chain finishes, into zv halves.
Measured dead ends (do not retry): gpsimd tensor_tensor (6.2us/op AND
inflates concurrent DVE tensor_tensor ~30% via the shared SBUF port);
DMA CCE multiply (NCC_IBIR077: only add-family supported in Copy mode);
chunked PE-scan (dBx cannot be produced transposed: it is elementwise in t);
exp/ln table fusion (compiler first-match puts Exp and Ln in different sets).
Run-to-run HW clock variance is ~+/-20% (observed 542..647us for identical
code); compare traces by instruction counts/avgs, not wall time alone.
"""

import sys

sys.path.insert(0, "/opt/trn_rl_repo")

from contextlib import ExitStack

import ml_dtypes
import numpy as np

import concourse.bacc as bacc
import concourse.bass as bass
import concourse.mybir as mybir
import concourse.tile as tile

BF = mybir.dt.bfloat16
F32 = mybir.dt.float32
AF = mybir.ActivationFunctionType
OP = mybir.AluOpType

B, L_FULL, IN_DIM = 8, 2048, 5
D_MODEL, OUT_DIM = 64, 64
N_LAYERS, D_INNER, N_STATE, DT_RANK, K = 2, 128, 16, 4, 4
T = 2 * N_LAYERS
N_CORES = 8

MM_F = 512  # max matmul free dim (one PSUM bank of f32)

# gpsimd tensor_tensor confirmed poisonous: 6.2us/op and inflates concurrent
# DVE tensor_tensor by ~30% (shared SBUF port). Keep everything off gpsimd.
MID_STATES = (3, 6, 9, 12)  # scan states after which front phases issue


def _mm(nc, out, lhsT, rhs, start=True, stop=True):
    F = rhs.shape[-1]
    for j in range(0, F, MM_F):
        e = min(j + MM_F, F)
        nc.tensor.matmul(out[:, j:e], lhsT, rhs[:, j:e], start=start, stop=stop)


def build_nc(L=L_FULL):
    nc = bacc.Bacc("TRN2", target_bir_lowering=False)
    H = L // 2

    # ---------------- DRAM I/O ----------------
    d_xT = nc.dram_tensor("xT", (IN_DIM, L), BF, kind="ExternalInput")
    d_Wemb = nc.dram_tensor("Wemb", (IN_DIM, D_MODEL), BF, kind="ExternalInput")
    d_bemb = nc.dram_tensor("bemb", (D_MODEL, 1), F32, kind="ExternalInput")
    d_peT = nc.dram_tensor("peT", (D_MODEL, L), BF, kind="ExternalInput")
    d_Wstat = nc.dram_tensor("Wstat", (D_MODEL, 2 * D_MODEL), BF, kind="ExternalInput")
    d_Win = nc.dram_tensor("Win", (D_MODEL, T * 2 * D_INNER), BF, kind="ExternalInput")
    d_beta = nc.dram_tensor("beta", (D_INNER, 2 * T), F32, kind="ExternalInput")
    d_cdiag = nc.dram_tensor("cdiag", (D_INNER, T * K * D_INNER), BF, kind="ExternalInput")
    d_bconv = nc.dram_tensor("bconv", (D_INNER, T), F32, kind="ExternalInput")
    d_Wxdt = nc.dram_tensor("Wxdt", (D_INNER, T * DT_RANK), BF, kind="ExternalInput")
    d_Wdt = nc.dram_tensor("Wdt", (DT_RANK, T * D_INNER), BF, kind="ExternalInput")
    d_bdt = nc.dram_tensor("bdt", (D_INNER, T), F32, kind="ExternalInput")
    d_WxB = nc.dram_tensor("WxB", (D_INNER, T * N_STATE), BF, kind="ExternalInput")
    d_WxC = nc.dram_tensor("WxC", (D_INNER, T * N_STATE), BF, kind="ExternalInput")
    d_sum4 = nc.dram_tensor("sum4", (D_INNER, 2 * 64), BF, kind="ExternalInput")
    d_dskd = nc.dram_tensor("dskd", (D_INNER, T * D_INNER), BF, kind="ExternalInput")
    d_Acol = nc.dram_tensor("Acol", (D_INNER, T * N_STATE), F32, kind="ExternalInput")
    d_Wout = nc.dram_tensor("Wout", (D_INNER, T * D_MODEL), BF, kind="ExternalInput")
    d_Wproj = nc.dram_tensor("Wproj", (2 * D_MODEL, OUT_DIM), F32, kind="ExternalInput")
    d_bproj = nc.dram_tensor("bproj", (OUT_DIM, 1), F32, kind="ExternalInput")
    d_out = nc.dram_tensor("out", (OUT_DIM, 1), F32, kind="ExternalOutput")

    with ExitStack() as ctx:
        tc = ctx.enter_context(tile.TileContext(nc))
        wp = ctx.enter_context(tc.tile_pool(name="weights", bufs=1))
        hp = ctx.enter_context(tc.tile_pool(name="hres", bufs=2))
        ap = ctx.enter_context(tc.tile_pool(name="acts", bufs=1))
        sp = ctx.enter_context(tc.tile_pool(name="scan", bufs=2))
        spr = ctx.enter_context(tc.tile_pool(name="rep", bufs=2))
        bcp = ctx.enter_context(tc.tile_pool(name="bc", bufs=1))
        sp3 = ctx.enter_context(tc.tile_pool(name="scan3", bufs=3))
        dp = ctx.enter_context(tc.tile_pool(name="dscr", bufs=1, space="DRAM"))
        psf = ctx.enter_context(tc.tile_pool(name="psF", bufs=1, space="PSUM"))
        psy = ctx.enter_context(tc.tile_pool(name="psY", bufs=1, space="PSUM"))

        # ---------------- load weights ----------------
        def wload(d, shape, dtype, nsplit=1):
            t = wp.tile(list(shape), dtype, tag="w_" + d.name)
            f = shape[1]
            step = (f + nsplit - 1) // nsplit
            for j in range(0, f, step):
                e = min(j + step, f)
                nc.sync.dma_start(t[:, j:e], d[:, j:e])
            return t

        s_Wemb = wload(d_Wemb, (IN_DIM, D_MODEL), BF)
        s_bemb = wload(d_bemb, (D_MODEL, 1), F32)

        # ---------------- embedding (inputs DMA'd before the heavy weights —
        # Win/cdiag are ~1MB — so layer-0's front isn't queued behind them) ---
        with tc.tile_pool(name="embin", bufs=1) as ep:
            s_xT = ep.tile([IN_DIM, L], BF, tag="xT")
            nc.sync.dma_start(s_xT[:], d_xT[:])
            s_peT = ep.tile([D_MODEL, L], BF, tag="peT")
            for j in (0, H):
                nc.sync.dma_start(s_peT[:, j:j + H], d_peT[:, j:j + H])
            s_Wstat = wload(d_Wstat, (D_MODEL, 2 * D_MODEL), BF)
            s_Win = wload(d_Win, (D_MODEL, T * 2 * D_INNER), BF)
            s_beta = wload(d_beta, (D_INNER, 2 * T), F32)
            s_cdiag = wload(d_cdiag, (D_INNER, T * K * D_INNER), BF, nsplit=2)
            s_bconv = wload(d_bconv, (D_INNER, T), F32)
            s_Wxdt = wload(d_Wxdt, (D_INNER, T * DT_RANK), BF)
            s_Wdt = wload(d_Wdt, (DT_RANK, T * D_INNER), BF)
            s_bdt = wload(d_bdt, (D_INNER, T), F32)
            s_WxB = wload(d_WxB, (D_INNER, T * N_STATE), BF)
            s_WxC = wload(d_WxC, (D_INNER, T * N_STATE), BF)
            s_eps = wp.tile([D_MODEL, 1], F32, tag="eps")
            nc.vector.memset(s_eps[:], 1e-5)
            s_Acol = wload(d_Acol, (D_INNER, T * N_STATE), F32)
            s_sum4 = wload(d_sum4, (D_INNER, 2 * 64), BF)
            s_dskd = wload(d_dskd, (D_INNER, T * D_INNER), BF)
            s_Wout = wload(d_Wout, (D_INNER, T * D_MODEL), BF)
            s_Wproj = wload(d_Wproj, (2 * D_MODEL, OUT_DIM), F32)
            s_bproj = wload(d_bproj, (OUT_DIM, 1), F32)
            h_f = hp.tile([D_MODEL, L], BF, tag="hf")
            for j in (0, H):
                eP = psf.tile([D_INNER, H], F32, tag="mmA")
                _mm(nc, eP[0:D_MODEL, :], s_Wemb[:], s_xT[:, j:j + H])
                nc.vector.scalar_tensor_tensor(
                    h_f[:, j:j + H], eP[0:D_MODEL, :], s_bemb[:], s_peT[:, j:j + H],
                    OP.add, OP.add)
            h_b = hp.tile([D_MODEL, L], BF, tag="hb")
            nc.scalar.activation(h_b[:], h_f[:, ::-1], AF.Identity)

        # ---------------- one mamba layer ----------------
        def front_phases(l, c, h_in):
            """LN + in-proj + conv + dt + u + bm/cm + scratch writes."""
            t = {}
            # --- LN: hm = (I - J/64) h ; var = J/64 hm^2 ---
            hmb = ap.tile([D_MODEL, L], BF, tag="hmb")
            inv = ap.tile([D_MODEL, L], BF, tag="inv")
            hmP = [psf.tile([D_INNER, H], F32, tag=tg, name="hmP" + tg) for tg in ("mmA", "mmB")]
            sq = [ap.tile([D_MODEL, H], BF, tag=tg, name="t" + tg) for tg in ("sq0", "sq1")]
            for i, j in enumerate((0, H)):
                _mm(nc, hmP[i][0:D_MODEL, :], s_Wstat[:, 0:D_MODEL], h_in[:, j:j + H])
            for i in (0, 1):
                nc.scalar.activation(sq[i][:], hmP[i][0:D_MODEL, :], AF.Square)
            for i, j in enumerate((0, H)):
                nc.scalar.activation(hmb[:, j:j + H], hmP[i][0:D_MODEL, :], AF.Identity)
            varP = [psf.tile([D_INNER, H], F32, tag=tg, name="varP" + tg) for tg in ("mmA", "mmB")]
            for i in (0, 1):
                _mm(nc, varP[i][0:D_MODEL, :], s_Wstat[:, D_MODEL:2 * D_MODEL], sq[i][:])
            for i, j in enumerate((0, H)):
                nc.scalar.activation(inv[:, j:j + H], varP[i][0:D_MODEL, :],
                                     AF.Abs_reciprocal_sqrt, bias=s_eps[:])
            hn = ap.tile([D_MODEL, L], BF, tag="hn")
            nc.vector.tensor_tensor(hn[:], hmb[:], inv[:], OP.mult)
            yield t

            # --- in-proj (xi into padded conv input, z -> silu) ---
            xi = ap.tile([D_INNER, L + K - 1], BF, tag="xi")
            nc.vector.memset(xi[:, 0:K - 1], 0.0)
            sz = ap.tile([D_INNER, L], BF, tag="sz" + c)
            w_in = s_Win[:, l * 2 * D_INNER:(l + 1) * 2 * D_INNER]
            for j in (0, H):
                xiP = psf.tile([D_INNER, H], F32, tag="mmA")
                _mm(nc, xiP, w_in[:, 0:D_INNER], hn[:, j:j + H])
                nc.scalar.activation(xi[:, K - 1 + j:K - 1 + j + H], xiP[:],
                                     AF.Identity, bias=s_beta[:, 2 * l:2 * l + 1])
                zP = psf.tile([D_INNER, H], F32, tag="mmB")
                _mm(nc, zP, w_in[:, D_INNER:2 * D_INNER], hn[:, j:j + H])
                nc.scalar.activation(sz[:, j:j + H], zP[:], AF.Silu,
                                     bias=s_beta[:, 2 * l + 1:2 * l + 2])
            # --- causal depthwise conv (4 diag matmuls) + silu ---
            xc = ap.tile([D_INNER, L], BF, tag="xc" + c)
            cP2 = [psf.tile([D_INNER, H], F32, tag=tg, name="cP" + tg) for tg in ("mmA", "mmB")]
            for k in range(K):  # k outer: stationary conv weight reused across halves
                dg = s_cdiag[:, (l * K + k) * D_INNER:(l * K + k + 1) * D_INNER]
                for i, j in enumerate((0, H)):
                    _mm(nc, cP2[i], dg, xi[:, j + k:j + k + H],
                        start=(k == 0), stop=(k == K - 1))
            for i, j in enumerate((0, H)):
                nc.scalar.activation(xc[:, j:j + H], cP2[i][:], AF.Silu,
                                     bias=s_bconv[:, l:l + 1])
            yield t
            # --- dt path -> delta = softplus(dt @ Wdt + b_dt) ---
            dt_bf = ap.tile([DT_RANK, L], BF, tag="dtbf")
            for j in (0, H):
                dtP = psf.tile([D_INNER, H], F32, tag="mmB")
                _mm(nc, dtP[0:DT_RANK, :], s_Wxdt[:, l * DT_RANK:(l + 1) * DT_RANK],
                    xc[:, j:j + H])
                nc.scalar.activation(dt_bf[:, j:j + H], dtP[0:DT_RANK, :], AF.Identity)
            delta = ap.tile([D_INNER, L], BF, tag="delta")
            dpP = [psf.tile([D_INNER, H], F32, tag=tg, name="dpP" + tg) for tg in ("mmA", "mmB")]
            dexp = [ap.tile([D_INNER, H], BF, tag=tg, name="t" + tg) for tg in ("dexp0", "dexp1")]
            for i, j in enumerate((0, H)):
                _mm(nc, dpP[i], s_Wdt[:, l * D_INNER:(l + 1) * D_INNER],
                    dt_bf[:, j:j + H])
            for i in (0, 1):
                nc.scalar.activation(dexp[i][:], dpP[i][:], AF.Exp,
                                     bias=s_bdt[:, l:l + 1])
            for i, j in enumerate((0, H)):
                nc.scalar.activation(delta[:, j:j + H], dexp[i][:], AF.Ln, bias=1.0)
            # --- u = delta * xc ---
            u = ap.tile([D_INNER, L], BF, tag="u")
            nc.vector.tensor_tensor(u[:], delta[:], xc[:], OP.mult)
            yield t
            # --- B/C compact rows (B at partition 0, C at 32: engine partition
            # base must be 0/32/64/96) + scratch DRAM write for replication ---
            sBC = ap.tile([32 + N_STATE, L], BF, tag="sbc" + c)
            for r0, w_all in ((0, s_WxB), (32, s_WxC)):
                for j in (0, H):
                    rP = psf.tile([D_INNER, H], F32, tag="mmB")
                    _mm(nc, rP[0:N_STATE, :],
                        w_all[:, l * N_STATE:(l + 1) * N_STATE], xc[:, j:j + H])
                    nc.scalar.activation(sBC[r0:r0 + N_STATE, j:j + H],
                                         rP[0:N_STATE, :], AF.Identity)
            scrBC = dp.tile([32 + N_STATE, L], BF, tag="scrBC" + c)
            for j in (0, H):
                nc.sync.dma_start(scrBC[:, j:j + H], sBC[:, j:j + H])
            # --- scratch DRAM writes for delta/u replication (32-row blocks so
            # the b=0 broadcast can start after the first quarter lands) ---
            scrD = dp.tile([D_INNER, L], BF, tag="scrD" + c)
            scrU = dp.tile([D_INNER, L], BF, tag="scrU" + c)
            for r in range(0, D_INNER, 32):
                nc.sync.dma_start(scrD[r:r + 32, :], delta[r:r + 32, :])
                nc.sync.dma_start(scrU[r:r + 32, :], u[r:r + 32, :])
            t.update(xc=xc, sz=sz, scrD=scrD, scrU=scrU, scrBC=scrBC,
                     h_in=h_in, l=l, c=c)
            yield t

        def scan_phase(t, mid_cb=None, post_cb=None):
            l, c = t["l"], t["c"]
            yaccP = psy.tile([D_INNER, L], F32, tag="yacc")
            t["yaccP"] = yaccP
            # D_skip * xc seeds the psum accumulation (diag matmuls per block)
            for blk in (0, 64):
                dgw = s_dskd[:, l * D_INNER + blk:l * D_INNER + blk + 64]
                for j in range(0, L, MM_F):
                    nc.tensor.matmul(yaccP[blk:blk + 64, j:j + MM_F], dgw,
                                     t["xc"][:, j:j + MM_F], start=True, stop=False,
                                     skip_group_check=True)
            # DMA issue order matters (one HW queue): state 0 needs dl0/ur0/bm0/
            # cm0 first; later coefficient tiles and the next b-block prefetch
            # behind them.
            def issue_dlur(b):
                dl = spr.tile([D_INNER, L], BF, tag="dl")
                nc.sync.dma_start(
                    dl[:], t["scrD"][32 * b:32 * b + 32, :].unsqueeze(1)
                    .broadcast_to((32, 4, L)))
                ur = spr.tile([D_INNER, L], BF, tag="ur")
                nc.sync.dma_start(
                    ur[:], t["scrU"][32 * b:32 * b + 32, :].unsqueeze(1)
                    .broadcast_to((32, 4, L)))
                return dl, ur

            cur = issue_dlur(0)
            bmt, cmt = [], []
            for nq in range(4):
                bq = bcp.tile([D_INNER, L], BF, tag=f"bm{nq}")
                nc.sync.dma_start(
                    bq[:], t["scrBC"][4 * nq:4 * nq + 4, :].unsqueeze(0)
                    .broadcast_to((32, 4, L)))
                bmt.append(bq)
                cq = bcp.tile([D_INNER, L], BF, tag=f"cm{nq}")
                nc.sync.dma_start(
                    cq[:], t["scrBC"][32 + 4 * nq:32 + 4 * nq + 4, :]
                    .unsqueeze(0).broadcast_to((32, 4, L)))
                cmt.append(cq)
            s = 0
            for b in range(4):
                dl, ur = cur
                if b < 3:
                    cur = issue_dlur(b + 1)
                for nq in range(4):
                    # dA/dBx triple-buffered so Act/DVE can run 2 states ahead
                    # of the scan and it never stalls on them
                    dA = sp3.tile([D_INNER, L], BF, tag="dA")
                    nc.scalar.activation(
                        dA[:], dl[:], AF.Exp,
                        scale=s_Acol[:, l * N_STATE + 4 * b + nq:
                                     l * N_STATE + 4 * b + nq + 1])
                    dBx = sp3.tile([D_INNER, L], BF, tag="dBx")
                    nc.vector.tensor_tensor(dBx[:], ur[:], bmt[nq][:], OP.mult)
                    hs = sp.tile([D_INNER, L], BF, tag="hs")
                    nc.vector.tensor_tensor_scan(hs[:], dA[:], dBx[:],
                                                 0.0, OP.mult, OP.add)
                    p = sp.tile([D_INNER, L], BF, tag="p")
                    nc.vector.tensor_tensor(p[:], cmt[nq][:], hs[:], OP.mult)
                    # PSUM base partition must be 0/32/64: write a 64-row block
                    # (base (b//2)*64); the parity-variant stationary routes this
                    # b's 32 rows, the other 32 columns are zero (accumulate +0).
                    base, par = (b // 2) * 64, b % 2
                    for j in range(0, L, MM_F):
                        nc.tensor.matmul(yaccP[base:base + 64, j:j + MM_F],
                                         s_sum4[:, par * 64:par * 64 + 64],
                                         p[:, j:j + MM_F],
                                         start=False, stop=(b % 2 == 1 and nq == 3),
                                         skip_group_check=True)
                    # previous unit's post is deferred to here (s==1) so this
                    # unit's dA0/dBx0/scan0 aren't queued behind its copies
                    if post_cb is not None and s == 1:
                        post_cb()
                    if mid_cb is not None and s in MID_STATES:
                        mid_cb()
                    s += 1

        def post_part(t, zslice=None):
            # --- postprocess: y = yacc (has D_skip term), gate, out-proj ---
            # zslice is used ONLY for the final (tail) unit: its h_out feeds
            # nothing but the mean-pool, so mean(h_in) issues up front
            # (overlapping the Act copies) and h_out is never materialized.
            # Applying this to the f-chain's last layer too was measured
            # WORSE (+2.3us of DVE work in the busy mid-scan region).
            l, c = t["l"], t["c"]
            yaccP = t["yaccP"]
            if zslice is not None:
                rh = ap.tile([D_MODEL, 1], F32, tag="rh" + c)
                nc.vector.tensor_reduce(rh[:], t["h_in"][:],
                                        axis=mybir.AxisListType.X, op=OP.add)
            yaccS = ap.tile([D_INNER, L], BF, tag="yac" + c)
            for j in (0, H):
                nc.scalar.activation(yaccS[:, j:j + H], yaccP[:, j:j + H], AF.Identity)
            yg = ap.tile([D_INNER, L], BF, tag="yg" + c)
            nc.vector.tensor_tensor(yg[:], yaccS[:], t["sz"], OP.mult)
            o_s = ap.tile([D_MODEL, L], BF, tag="os" + c)
            for j in (0, H):
                oP = psf.tile([D_INNER, H], F32, tag="mmA")
                _mm(nc, oP[0:D_MODEL, :], s_Wout[:, l * D_MODEL:(l + 1) * D_MODEL],
                    yg[:, j:j + H])
                nc.scalar.activation(o_s[:, j:j + H], oP[0:D_MODEL, :], AF.Identity)
            if zslice is not None:
                ro = ap.tile([D_MODEL, 1], F32, tag="ro" + c)
                nc.vector.tensor_reduce(ro[:], o_s[:],
                                        axis=mybir.AxisListType.X, op=OP.add)
                nc.vector.tensor_tensor(zv[zslice[0]:zslice[1], :], rh[:], ro[:],
                                        OP.add)
                return None
            h_out = hp.tile([D_MODEL, L], BF, tag="h" + c)
            nc.vector.tensor_tensor(h_out[:], t["h_in"][:], o_s[:], OP.add)
            return h_out

        # ---------------- run the 4 layer-units, fronts interleaved ----------
        units = [(0, "f"), (N_LAYERS, "b"), (1, "f"), (N_LAYERS + 1, "b")]
        hcur = {"f": h_f, "b": h_b}

        def run_all(gen):
            t = None
            for t in gen:
                pass
            return t

        zv = ap.tile([2 * D_MODEL, 1], F32, tag="zv")
        ctx_next = run_all(front_phases(*units[0], hcur[units[0][1]]))
        prev_t = None
        for i in range(len(units)):
            t_cur = ctx_next
            holder = {}

            def mk_cb(idx, holder):
                if idx + 1 >= len(units):
                    return None
                ln, cn = units[idx + 1]
                state = {}

                def cb():
                    # generator built lazily: hcur[cn] is updated by the
                    # deferred post (s==1) before the first front phase (s==3)
                    if "gen" not in state:
                        state["gen"] = front_phases(ln, cn, hcur[cn])
                    try:
                        holder["ctx"] = next(state["gen"])
                    except StopIteration:
                        pass
                return cb

            def mk_post(pt, pidx):
                if pt is None:
                    return None

                def cb():
                    h_new = post_part(pt)
                    hcur[pt["c"]] = h_new
                    if pidx == 2:  # f-chain done: mean-pool it now
                        nc.vector.tensor_reduce(
                            zv[0:D_MODEL, :], h_new[:],
                            axis=mybir.AxisListType.X, op=OP.add)
                return cb

            scan_phase(t_cur, mk_cb(i, holder), mk_post(prev_t, i - 1))
            prev_t = t_cur
            ctx_next = holder.get("ctx")
        post_part(prev_t, zslice=(D_MODEL, 2 * D_MODEL))

        # ---------------- head ----------------
        oP = psf.tile([D_INNER, 1], F32, tag="mmB")
        nc.tensor.matmul(oP[0:OUT_DIM, :], s_Wproj[:], zv[:])
        ofin = ap.tile([OUT_DIM, 1], F32, tag="ofin")
        nc.scalar.activation(ofin[:], oP[0:OUT_DIM, :], AF.Identity,
                             bias=s_bproj[:])
        nc.sync.dma_start(d_out[:], ofin[:])

    return nc


def prep_inputs(inputs, L=L_FULL):
    bf = ml_dtypes.bfloat16
    f32 = np.float32
    g = {k: np.asarray(v) for k, v in inputs.items()}
    W_in, W_conv, W_x, W_dt = g["W_in"], g["W_conv"], g["W_x"], g["W_dt"]
    ln_w, ln_b = g["ln_w"], g["ln_b"]

    Win = np.concatenate([W_in[l] * ln_w[l][:, None] for l in range(T)], axis=1)
    beta = np.stack([ln_b[l] @ W_in[l] for l in range(T)], 0)
    beta_blob = np.zeros((D_INNER, 2 * T), f32)
    for l in range(T):
        beta_blob[:, 2 * l] = beta[l, :D_INNER]
        beta_blob[:, 2 * l + 1] = beta[l, D_INNER:]
    cdiag = np.zeros((D_INNER, T * K * D_INNER), f32)
    for l in range(T):
        for k in range(K):
            blk = (l * K + k) * D_INNER
            cdiag[np.arange(D_INNER), blk + np.arange(D_INNER)] = W_conv[l, :, 0, k]
    Wxdt = np.concatenate([W_x[l][:, :DT_RANK] for l in range(T)], axis=1)
    Wdt = np.concatenate([W_dt[l] for l in range(T)], axis=1)
    WxB = np.concatenate(
        [W_x[l][:, DT_RANK:DT_RANK + N_STATE] for l in range(T)], axis=1)
    WxC = np.concatenate(
        [W_x[l][:, DT_RANK + N_STATE:] for l in range(T)], axis=1)
    # (d32, n4) partition layout: p = d32*4 + j; sum over the 4 j-partitions.
    # Two 64-col variants: even b -> rows land in cols 0-31, odd b -> cols 32-63.
    sum4 = np.zeros((D_INNER, 2 * 64), f32)
    for i in range(32):
        for j in range(4):
            sum4[4 * i + j, i] = 1.0
            sum4[4 * i + j, 64 + 32 + i] = 1.0
    # diag(D_skip) blocks for the yacc seed matmuls
    dskd = np.zeros((D_INNER, T * D_INNER), f32)
    for l in range(T):
        for blk in (0, 64):
            for q in range(64):
                dskd[blk + q, l * D_INNER + blk + q] = g["D_skip"][l][blk + q]
    A = -np.exp(g["A_log"])
    Acol = np.zeros((D_INNER, T * N_STATE), f32)
    pidx = np.arange(D_INNER)
    for l in range(T):
        for b in range(4):
            for nq in range(4):
                Acol[:, l * N_STATE + 4 * b + nq] = A[l][32 * b + pidx // 4,
                                                         4 * nq + pidx % 4]
    Wout = np.concatenate([g["W_out"][l] for l in range(T)], axis=1)
    # LN stats weights: [I - J/64 | J/64]
    Wstat = np.zeros((D_MODEL, 2 * D_MODEL), f32)
    Wstat[:, 0:D_MODEL] = np.eye(D_MODEL) - 1.0 / D_MODEL
    Wstat[:, D_MODEL:] = 1.0 / D_MODEL

    shared = {
        "Wemb": g["W_emb"].astype(bf),
        "bemb": g["b_emb"].reshape(D_MODEL, 1).astype(f32),
        "peT": np.ascontiguousarray(g["pe"][:L].T).astype(bf),
        "Wstat": Wstat.astype(bf),
        "Win": Win.astype(bf),
        "beta": beta_blob,
        "cdiag": cdiag.astype(bf),
        "bconv": np.ascontiguousarray(g["b_conv"].T).astype(f32),
        "Wxdt": Wxdt.astype(bf),
        "Wdt": Wdt.astype(bf),
        "bdt": np.ascontiguousarray(g["b_dt"].T).astype(f32),
        "WxB": WxB.astype(bf),
        "WxC": WxC.astype(bf),
        "sum4": sum4.astype(bf),
        "dskd": dskd.astype(bf),
        "Acol": Acol.astype(f32),
        "Wout": Wout.astype(bf),
        "Wproj": (g["W_proj"] / L).astype(f32),
        "bproj": g["b_proj"].reshape(OUT_DIM, 1).astype(f32),
    }
    in_maps = []
    for c in range(B):
        m = dict(shared)
        m["xT"] = np.ascontiguousarray(g["x"][c, :L].T).astype(bf)
        in_maps.append(m)
    return in_maps


_CACHE = {}


def kernel(**inputs):
    if "nc" not in _CACHE:
        _CACHE["nc"] = build_nc()
        _CACHE["nc"].finalize()
    nc = _CACHE["nc"]
    in_maps = prep_inputs(inputs)
    from concourse.bass_utils import run_bass_kernel_spmd
    res = run_bass_kernel_spmd(nc, in_maps, core_ids=list(range(N_CORES)))
    out = np.stack([np.asarray(res.results[c]["out"]).reshape(OUT_DIM)
                    for c in range(N_CORES)], axis=0)
    return out.astype(np.float32)



# revision 58
# speedup vs baseline: 1.0487x; 1.0027x over previous
"""Trainium2 Bass kernel for nn_MicroBiMambaBackbone (v3).

Data-parallel over batch (B=8 -> 8 cores, 1 sample/core).
Measured (clean clock state): 539us, rel err 4.1e-4. Vector engine ~99%
occupied = the wall: 282us scan (tensor_tensor_scan is 0.5 elem/cy/lane,
inherent) + 174us dBx/pmult tensor_tensor with no legal alternate engine.
Remaining vector idle is ~92us: ~16us fixed tail (NOTIFY/teardown), ~50us
layer-0 front latency (serial LN->conv->dt->delta chain, nothing for DVE),
rest mid-scan Act/DMA waits.
Changes vs v2 (~580us -> ~539us; vector engine now ~99% occupied):
  - Embedding inputs + early-front weights DMA'd before the ~1MB of late
    weights; dA/dBx triple-buffered so Act runs 2 states ahead of the scan.
  - Scan partition layout (d8,n16) -> (d32,n4): delta/u replicate x4 instead
    of x16, B/C broadcast from compact (16,L) DRAM scratch instead of
    PE-tiled production. DMA traffic ~17MB -> ~9MB per layer (DMA engine
    union-active 475us -> 156us), which removed DMA-gated scan stalls.
  - n-reduction via sum4 stationaries (two 64-col parity variants since
    PSUM matmul base partition must be 0/32/64).
  - Full-length (FD=2048) tensor_tensor_scan per state tile (was 2 halves).
  - DMA issue order: dl0/ur0 before the bm/cm prefetch, dl/ur one b-block
    ahead; scratch written in 32-row blocks so b=0 broadcast starts early.
  - Mean-pool reduces issue as soon as each chain finishes, into zv halves.
Measured dead ends (do not retry): gpsimd tensor_tensor (6.2us/op AND
inflates concurrent DVE tensor_tensor ~30% via the shared SBUF port);
DMA CCE multiply (NCC_IBIR077: only add-family supported in Copy mode);
chunked PE-scan (dBx cannot be produced transposed: it is elementwise in t);
exp/ln table fusion (compiler first-match puts Exp and Ln in different sets).
Run-to-run HW clock variance is ~+/-20% (observed 542..647us for identical
code); compare traces by instruction counts/avgs, not wall time alone.
"""

import sys

sys.path.insert(0, "/opt/trn_rl_repo")

from contextlib import ExitStack

import ml_dtypes
import numpy as np

import concourse.bacc as bacc
import concourse.bass as bass
import concourse.mybir as mybir
import concourse.tile as tile

BF = mybir.dt.bfloat16
F32 = mybir.dt.float32
AF = mybir.ActivationFunctionType
OP = mybir.AluOpType

B, L_FULL, IN_DIM = 8, 2048, 5
D_MODEL, OUT_DIM = 64, 64
N_LAYERS, D_INNER, N_STATE, DT_RANK, K = 2, 128, 16, 4, 4
T = 2 * N_LAYERS
N_CORES = 8

MM_F = 512  # max matmul free dim (one PSUM bank of f32)

# gpsimd tensor_tensor confirmed poisonous: 6.2us/op and inflates concurrent
# DVE tensor_tensor by ~30% (shared SBUF port). Keep everything off gpsimd.
MID_STATES = (3, 6, 9, 12)  # scan states after which front phases issue


def _mm(nc, out, lhsT, rhs, start=True, stop=True):
    F = rhs.shape[-1]
    for j in range(0, F, MM_F):
        e = min(j + MM_F, F)
        nc.tensor.matmul(out[:, j:e], lhsT, rhs[:, j:e], start=start, stop=stop)


def build_nc(L=L_FULL):
    nc = bacc.Bacc("TRN2", target_bir_lowering=False)
    H = L // 2

    # ---------------- DRAM I/O ----------------
    d_xT = nc.dram_tensor("xT", (IN_DIM, L), BF, kind="ExternalInput")
    d_Wemb = nc.dram_tensor("Wemb", (IN_DIM, D_MODEL), BF, kind="ExternalInput")
    d_bemb = nc.dram_tensor("bemb", (D_MODEL, 1), F32, kind="ExternalInput")
    d_peT = nc.dram_tensor("peT", (D_MODEL, L), BF, kind="ExternalInput")
    d_Wstat = nc.dram_tensor("Wstat", (D_MODEL, 2 * D_MODEL), BF, kind="ExternalInput")
    d_Win = nc.dram_tensor("Win", (D_MODEL, T * 2 * D_INNER), BF, kind="ExternalInput")
    d_beta = nc.dram_tensor("beta", (D_INNER, 2 * T), F32, kind="ExternalInput")
    d_cdiag = nc.dram_tensor("cdiag", (D_INNER, T * K * D_INNER), BF, kind="ExternalInput")
    d_bconv = nc.dram_tensor("bconv", (D_INNER, T), F32, kind="ExternalInput")
    d_Wxdt = nc.dram_tensor("Wxdt", (D_INNER, T * DT_RANK), BF, kind="ExternalInput")
    d_Wdt = nc.dram_tensor("Wdt", (DT_RANK, T * D_INNER), BF, kind="ExternalInput")
    d_bdt = nc.dram_tensor("bdt", (D_INNER, T), F32, kind="ExternalInput")
    d_WxB = nc.dram_tensor("WxB", (D_INNER, T * N_STATE), BF, kind="ExternalInput")
    d_WxC = nc.dram_tensor("WxC", (D_INNER, T * N_STATE), BF, kind="ExternalInput")
    d_sum4 = nc.dram_tensor("sum4", (D_INNER, 2 * 64), BF, kind="ExternalInput")
    d_dskd = nc.dram_tensor("dskd", (D_INNER, T * D_INNER), BF, kind="ExternalInput")
    d_Acol = nc.dram_tensor("Acol", (D_INNER, T * N_STATE), F32, kind="ExternalInput")
    d_Wout = nc.dram_tensor("Wout", (D_INNER, T * D_MODEL), BF, kind="ExternalInput")
    d_Wproj = nc.dram_tensor("Wproj", (2 * D_MODEL, OUT_DIM), F32, kind="ExternalInput")
    d_bproj = nc.dram_tensor("bproj", (OUT_DIM, 1), F32, kind="ExternalInput")
    d_out = nc.dram_tensor("out", (OUT_DIM, 1), F32, kind="ExternalOutput")

    with ExitStack() as ctx:
        tc = ctx.enter_context(tile.TileContext(nc))
        wp = ctx.enter_context(tc.tile_pool(name="weights", bufs=1))
        hp = ctx.enter_context(tc.tile_pool(name="hres", bufs=2))
        ap = ctx.enter_context(tc.tile_pool(name="acts", bufs=1))
        sp = ctx.enter_context(tc.tile_pool(name="scan", bufs=2))
        spr = ctx.enter_context(tc.tile_pool(name="rep", bufs=2))
        bcp = ctx.enter_context(tc.tile_pool(name="bc", bufs=1))
        sp3 = ctx.enter_context(tc.tile_pool(name="scan3", bufs=3))
        dp = ctx.enter_context(tc.tile_pool(name="dscr", bufs=1, space="DRAM"))
        psf = ctx.enter_context(tc.tile_pool(name="psF", bufs=1, space="PSUM"))
        psy = ctx.enter_context(tc.tile_pool(name="psY", bufs=1, space="PSUM"))

        # ---------------- load weights ----------------
        def wload(d, shape, dtype, nsplit=1):
            t = wp.tile(list(shape), dtype, tag="w_" + d.name)
            f = shape[1]
            step = (f + nsplit - 1) // nsplit
            for j in range(0, f, step):
                e = min(j + step, f)
                nc.sync.dma_start(t[:, j:e], d[:, j:e])
            return t

        s_Wemb = wload(d_Wemb, (IN_DIM, D_MODEL), BF)
        s_bemb = wload(d_bemb, (D_MODEL, 1), F32)

        # ---------------- embedding (inputs DMA'd before the heavy weights —
        # Win/cdiag are ~1MB — so layer-0's front isn't queued behind them) ---
        with tc.tile_pool(name="embin", bufs=1) as ep:
            s_xT = ep.tile([IN_DIM, L], BF, tag="xT")
            nc.sync.dma_start(s_xT[:], d_xT[:])
            s_peT = ep.tile([D_MODEL, L], BF, tag="peT")
            for j in (0, H):
                nc.sync.dma_start(s_peT[:, j:j + H], d_peT[:, j:j + H])
            s_Wstat = wload(d_Wstat, (D_MODEL, 2 * D_MODEL), BF)
            s_Win = wload(d_Win, (D_MODEL, T * 2 * D_INNER), BF)
            s_beta = wload(d_beta, (D_INNER, 2 * T), F32)
            s_cdiag = wload(d_cdiag, (D_INNER, T * K * D_INNER), BF, nsplit=2)
            s_bconv = wload(d_bconv, (D_INNER, T), F32)
            s_Wxdt = wload(d_Wxdt, (D_INNER, T * DT_RANK), BF)
            s_Wdt = wload(d_Wdt, (DT_RANK, T * D_INNER), BF)
            s_bdt = wload(d_bdt, (D_INNER, T), F32)
            s_WxB = wload(d_WxB, (D_INNER, T * N_STATE), BF)
            s_WxC = wload(d_WxC, (D_INNER, T * N_STATE), BF)
            s_eps = wp.tile([D_MODEL, 1], F32, tag="eps")
            nc.vector.memset(s_eps[:], 1e-5)
            s_Acol = wload(d_Acol, (D_INNER, T * N_STATE), F32)
            s_sum4 = wload(d_sum4, (D_INNER, 2 * 64), BF)
            s_dskd = wload(d_dskd, (D_INNER, T * D_INNER), BF)
            s_Wout = wload(d_Wout, (D_INNER, T * D_MODEL), BF)
            s_Wproj = wload(d_Wproj, (2 * D_MODEL, OUT_DIM), F32)
            s_bproj = wload(d_bproj, (OUT_DIM, 1), F32)
            h_f = hp.tile([D_MODEL, L], BF, tag="hf")
            for j in (0, H):
                eP = psf.tile([D_INNER, H], F32, tag="mmA")
                _mm(nc, eP[0:D_MODEL, :], s_Wemb[:], s_xT[:, j:j + H])
                nc.vector.scalar_tensor_tensor(
                    h_f[:, j:j + H], eP[0:D_MODEL, :], s_bemb[:], s_peT[:, j:j + H],
                    OP.add, OP.add)
            h_b = hp.tile([D_MODEL, L], BF, tag="hb")
            nc.scalar.activation(h_b[:], h_f[:, ::-1], AF.Identity)

        # ---------------- one mamba layer ----------------
        def front_phases(l, c, h_in):
            """LN + in-proj + conv + dt + u + bm/cm + scratch writes."""
            t = {}
            # --- LN: hm = (I - J/64) h ; var = J/64 hm^2 ---
            hmb = ap.tile([D_MODEL, L], BF, tag="hmb")
            inv = ap.tile([D_MODEL, L], BF, tag="inv")
            hmP = [psf.tile([D_INNER, H], F32, tag=tg, name="hmP" + tg) for tg in ("mmA", "mmB")]
            sq = [ap.tile([D_MODEL, H], BF, tag=tg, name="t" + tg) for tg in ("sq0", "sq1")]
            for i, j in enumerate((0, H)):
                _mm(nc, hmP[i][0:D_MODEL, :], s_Wstat[:, 0:D_MODEL], h_in[:, j:j + H])
            for i in (0, 1):
                nc.scalar.activation(sq[i][:], hmP[i][0:D_MODEL, :], AF.Square)
            for i, j in enumerate((0, H)):
                nc.scalar.activation(hmb[:, j:j + H], hmP[i][0:D_MODEL, :], AF.Identity)
            varP = [psf.tile([D_INNER, H], F32, tag=tg, name="varP" + tg) for tg in ("mmA", "mmB")]
            for i in (0, 1):
                _mm(nc, varP[i][0:D_MODEL, :], s_Wstat[:, D_MODEL:2 * D_MODEL], sq[i][:])
            for i, j in enumerate((0, H)):
                nc.scalar.activation(inv[:, j:j + H], varP[i][0:D_MODEL, :],
                                     AF.Abs_reciprocal_sqrt, bias=s_eps[:])
            hn = ap.tile([D_MODEL, L], BF, tag="hn")
            nc.vector.tensor_tensor(hn[:], hmb[:], inv[:], OP.mult)
            yield t

            # --- in-proj (xi into padded conv input, z -> silu) ---
            xi = ap.tile([D_INNER, L + K - 1], BF, tag="xi")
            nc.vector.memset(xi[:, 0:K - 1], 0.0)
            sz = ap.tile([D_INNER, L], BF, tag="sz" + c)
            w_in = s_Win[:, l * 2 * D_INNER:(l + 1) * 2 * D_INNER]
            for j in (0, H):
                xiP = psf.tile([D_INNER, H], F32, tag="mmA")
                _mm(nc, xiP, w_in[:, 0:D_INNER], hn[:, j:j + H])
                nc.scalar.activation(xi[:, K - 1 + j:K - 1 + j + H], xiP[:],
                                     AF.Identity, bias=s_beta[:, 2 * l:2 * l + 1])
                zP = psf.tile([D_INNER, H], F32, tag="mmB")
                _mm(nc, zP, w_in[:, D_INNER:2 * D_INNER], hn[:, j:j + H])
                nc.scalar.activation(sz[:, j:j + H], zP[:], AF.Silu,
                                     bias=s_beta[:, 2 * l + 1:2 * l + 2])
            # --- causal depthwise conv (4 diag matmuls) + silu ---
            xc = ap.tile([D_INNER, L], BF, tag="xc" + c)
            cP2 = [psf.tile([D_INNER, H], F32, tag=tg, name="cP" + tg) for tg in ("mmA", "mmB")]
            for k in range(K):  # k outer: stationary conv weight reused across halves
                dg = s_cdiag[:, (l * K + k) * D_INNER:(l * K + k + 1) * D_INNER]
                for i, j in enumerate((0, H)):
                    _mm(nc, cP2[i], dg, xi[:, j + k:j + k + H],
                        start=(k == 0), stop=(k == K - 1))
            for i, j in enumerate((0, H)):
                nc.scalar.activation(xc[:, j:j + H], cP2[i][:], AF.Silu,
                                     bias=s_bconv[:, l:l + 1])
            yield t
            # --- dt path -> delta = softplus(dt @ Wdt + b_dt) ---
            dt_bf = ap.tile([DT_RANK, L], BF, tag="dtbf")
            for j in (0, H):
                dtP = psf.tile([D_INNER, H], F32, tag="mmB")
                _mm(nc, dtP[0:DT_RANK, :], s_Wxdt[:, l * DT_RANK:(l + 1) * DT_RANK],
                    xc[:, j:j + H])
                nc.scalar.activation(dt_bf[:, j:j + H], dtP[0:DT_RANK, :], AF.Identity)
            delta = ap.tile([D_INNER, L], BF, tag="delta")
            dpP = [psf.tile([D_INNER, H], F32, tag=tg, name="dpP" + tg) for tg in ("mmA", "mmB")]
            dexp = [ap.tile([D_INNER, H], BF, tag=tg, name="t" + tg) for tg in ("dexp0", "dexp1")]
            for i, j in enumerate((0, H)):
                _mm(nc, dpP[i], s_Wdt[:, l * D_INNER:(l + 1) * D_INNER],
                    dt_bf[:, j:j + H])
            for i in (0, 1):
                nc.scalar.activation(dexp[i][:], dpP[i][:], AF.Exp,
                                     bias=s_bdt[:, l:l + 1])
            for i, j in enumerate((0, H)):
                nc.scalar.activation(delta[:, j:j + H], dexp[i][:], AF.Ln, bias=1.0)
            # --- u = delta * xc ---
            u = ap.tile([D_INNER, L], BF, tag="u")
            nc.vector.tensor_tensor(u[:], delta[:], xc[:], OP.mult)
            yield t
            # --- B/C compact rows (B at partition 0, C at 32: engine partition
            # base must be 0/32/64/96) + scratch DRAM write for replication ---
            sBC = ap.tile([32 + N_STATE, L], BF, tag="sbc" + c)
            for r0, w_all in ((0, s_WxB), (32, s_WxC)):
                for j in (0, H):
                    rP = psf.tile([D_INNER, H], F32, tag="mmB")
                    _mm(nc, rP[0:N_STATE, :],
                        w_all[:, l * N_STATE:(l + 1) * N_STATE], xc[:, j:j + H])
                    nc.scalar.activation(sBC[r0:r0 + N_STATE, j:j + H],
                                         rP[0:N_STATE, :], AF.Identity)
            scrBC = dp.tile([32 + N_STATE, L], BF, tag="scrBC" + c)
            for j in (0, H):
                nc.sync.dma_start(scrBC[:, j:j + H], sBC[:, j:j + H])
            # --- scratch DRAM writes for delta/u replication (32-row blocks so
            # the b=0 broadcast can start after the first quarter lands) ---
            scrD = dp.tile([D_INNER, L], BF, tag="scrD" + c)
            scrU = dp.tile([D_INNER, L], BF, tag="scrU" + c)
            for r in range(0, D_INNER, 32):
                nc.sync.dma_start(scrD[r:r + 32, :], delta[r:r + 32, :])
                nc.sync.dma_start(scrU[r:r + 32, :], u[r:r + 32, :])
            t.update(xc=xc, sz=sz, scrD=scrD, scrU=scrU, scrBC=scrBC,
                     h_in=h_in, l=l, c=c)
            yield t

        def scan_phase(t, mid_cb=None, post_cb=None):
            l, c = t["l"], t["c"]
            yaccP = psy.tile([D_INNER, L], F32, tag="yacc")
            t["yaccP"] = yaccP
            # D_skip * xc seeds the psum accumulation (diag matmuls per block)
            for blk in (0, 64):
                dgw = s_dskd[:, l * D_INNER + blk:l * D_INNER + blk + 64]
                for j in range(0, L, MM_F):
                    nc.tensor.matmul(yaccP[blk:blk + 64, j:j + MM_F], dgw,
                                     t["xc"][:, j:j + MM_F], start=True, stop=False,
                                     skip_group_check=True)
            # DMA issue order matters (one HW queue): state 0 needs dl0/ur0/bm0/
            # cm0 first; later coefficient tiles and the next b-block prefetch
            # behind them.
            def issue_dlur(b):
                dl = spr.tile([D_INNER, L], BF, tag="dl")
                nc.sync.dma_start(
                    dl[:], t["scrD"][32 * b:32 * b + 32, :].unsqueeze(1)
                    .broadcast_to((32, 4, L)))
                ur = spr.tile([D_INNER, L], BF, tag="ur")
                nc.sync.dma_start(
                    ur[:], t["scrU"][32 * b:32 * b + 32, :].unsqueeze(1)
                    .broadcast_to((32, 4, L)))
                return dl, ur

            cur = issue_dlur(0)
            bmt, cmt = [], []
            for nq in range(4):
                bq = bcp.tile([D_INNER, L], BF, tag=f"bm{nq}")
                nc.sync.dma_start(
                    bq[:], t["scrBC"][4 * nq:4 * nq + 4, :].unsqueeze(0)
                    .broadcast_to((32, 4, L)))
                bmt.append(bq)
                cq = bcp.tile([D_INNER, L], BF, tag=f"cm{nq}")
                nc.sync.dma_start(
                    cq[:], t["scrBC"][32 + 4 * nq:32 + 4 * nq + 4, :]
                    .unsqueeze(0).broadcast_to((32, 4, L)))
                cmt.append(cq)
            s = 0
            for b in range(4):
                dl, ur = cur
                if b < 3:
                    cur = issue_dlur(b + 1)
                for nq in range(4):
                    # dA/dBx triple-buffered so Act/DVE can run 2 states ahead
                    # of the scan and it never stalls on them
                    dA = sp3.tile([D_INNER, L], BF, tag="dA")
                    nc.scalar.activation(
                        dA[:], dl[:], AF.Exp,
                        scale=s_Acol[:, l * N_STATE + 4 * b + nq:
                                     l * N_STATE + 4 * b + nq + 1])
                    dBx = sp3.tile([D_INNER, L], BF, tag="dBx")
                    nc.vector.tensor_tensor(dBx[:], ur[:], bmt[nq][:], OP.mult)
                    hs = sp.tile([D_INNER, L], BF, tag="hs")
                    nc.vector.tensor_tensor_scan(hs[:], dA[:], dBx[:],
                                                 0.0, OP.mult, OP.add)
                    p = sp.tile([D_INNER, L], BF, tag="p")
                    nc.vector.tensor_tensor(p[:], cmt[nq][:], hs[:], OP.mult)
                    # PSUM base partition must be 0/32/64: write a 64-row block
                    # (base (b//2)*64); the parity-variant stationary routes this
                    # b's 32 rows, the other 32 columns are zero (accumulate +0).
                    base, par = (b // 2) * 64, b % 2
                    for j in range(0, L, MM_F):
                        nc.tensor.matmul(yaccP[base:base + 64, j:j + MM_F],
                                         s_sum4[:, par * 64:par * 64 + 64],
                                         p[:, j:j + MM_F],
                                         start=False, stop=(b % 2 == 1 and nq == 3),
                                         skip_group_check=True)
                    # previous unit's post is deferred to here (s==1) so this
                    # unit's dA0/dBx0/scan0 aren't queued behind its copies
                    if post_cb is not None and s == 1:
                        post_cb()
                    if mid_cb is not None and s in MID_STATES:
                        mid_cb()
                    s += 1

        def post_part(t):
            # --- postprocess: y = yacc (has D_skip term), gate, out-proj ---
            l, c = t["l"], t["c"]
            yaccP = t["yaccP"]
            yaccS = ap.tile([D_INNER, L], BF, tag="yac" + c)
            for j in (0, H):
                nc.scalar.activation(yaccS[:, j:j + H], yaccP[:, j:j + H], AF.Identity)
            yg = ap.tile([D_INNER, L], BF, tag="yg" + c)
            nc.vector.tensor_tensor(yg[:], yaccS[:], t["sz"], OP.mult)
            o_s = ap.tile([D_MODEL, L], BF, tag="os" + c)
            for j in (0, H):
                oP = psf.tile([D_INNER, H], F32, tag="mmA")
                _mm(nc, oP[0:D_MODEL, :], s_Wout[:, l * D_MODEL:(l + 1) * D_MODEL],
                    yg[:, j:j + H])
                nc.scalar.activation(o_s[:, j:j + H], oP[0:D_MODEL, :], AF.Identity)
            h_out = hp.tile([D_MODEL, L], BF, tag="h" + c)
            nc.vector.tensor_tensor(h_out[:], t["h_in"][:], o_s[:], OP.add)
            return h_out

        # ---------------- run the 4 layer-units, fronts interleaved ----------
        units = [(0, "f"), (N_LAYERS, "b"), (1, "f"), (N_LAYERS + 1, "b")]
        hcur = {"f": h_f, "b": h_b}

        def run_all(gen):
            t = None
            for t in gen:
                pass
            return t

        zv = ap.tile([2 * D_MODEL, 1], F32, tag="zv")
        ctx_next = run_all(front_phases(*units[0], hcur[units[0][1]]))
        prev_t = None
        for i in range(len(units)):
            t_cur = ctx_next
            holder = {}

            def mk_cb(idx, holder):
                if idx + 1 >= len(units):
                    return None
                ln, cn = units[idx + 1]
                state = {}

                def cb():
                    # generator built lazily: hcur[cn] is updated by the
                    # deferred post (s==1) before the first front phase (s==3)
                    if "gen" not in state:
                        state["gen"] = front_phases(ln, cn, hcur[cn])
                    try:
                        holder["ctx"] = next(state["gen"])
                    except StopIteration:
                        pass
                return cb

            def mk_post(pt, pidx):
                if pt is None:
                    return None

                def cb():
                    h_new = post_part(pt)
                    hcur[pt["c"]] = h_new
                    if pidx == 2:  # f-chain done: mean-pool it now
                        nc.vector.tensor_reduce(
                            zv[0:D_MODEL, :], h_new[:],
                            axis=mybir.AxisListType.X, op=OP.add)
                return cb

            scan_phase(t_cur, mk_cb(i, holder), mk_post(prev_t, i - 1))
            prev_t = t_cur
            ctx_next = holder.get("ctx")
        h_last = post_part(prev_t)
        hcur[prev_t["c"]] = h_last
        nc.vector.tensor_reduce(zv[D_MODEL:2 * D_MODEL, :], h_last[:],
                                axis=mybir.AxisListType.X, op=OP.add)

        # ---------------- head ----------------
        oP = psf.tile([D_INNER, 1], F32, tag="mmB")
        nc.tensor.matmul(oP[0:OUT_DIM, :], s_Wproj[:], zv[:])
        ofin = ap.tile([OUT_DIM, 1], F32, tag="ofin")
        nc.scalar.activation(ofin[:], oP[0:OUT_DIM, :], AF.Identity,
                             bias=s_bproj[:])
        nc.sync.dma_start(d_out[:], ofin[:])

    return nc


def prep_inputs(inputs, L=L_FULL):
    bf = ml_dtypes.bfloat16
    f32 = np.float32
    g = {k: np.asarray(v) for k, v in inputs.items()}
    W_in, W_conv, W_x, W_dt = g["W_in"], g["W_conv"], g["W_x"], g["W_dt"]
    ln_w, ln_b = g["ln_w"], g["ln_b"]

    Win = np.concatenate([W_in[l] * ln_w[l][:, None] for l in range(T)], axis=1)
    beta = np.stack([ln_b[l] @ W_in[l] for l in range(T)], 0)
    beta_blob = np.zeros((D_INNER, 2 * T), f32)
    for l in range(T):
        beta_blob[:, 2 * l] = beta[l, :D_INNER]
        beta_blob[:, 2 * l + 1] = beta[l, D_INNER:]
    cdiag = np.zeros((D_INNER, T * K * D_INNER), f32)
    for l in range(T):
        for k in range(K):
            blk = (l * K + k) * D_INNER
            cdiag[np.arange(D_INNER), blk + np.arange(D_INNER)] = W_conv[l, :, 0, k]
    Wxdt = np.concatenate([W_x[l][:, :DT_RANK] for l in range(T)], axis=1)
    Wdt = np.concatenate([W_dt[l] for l in range(T)], axis=1)
    WxB = np.concatenate(
        [W_x[l][:, DT_RANK:DT_RANK + N_STATE] for l in range(T)], axis=1)
    WxC = np.concatenate(
        [W_x[l][:, DT_RANK + N_STATE:] for l in range(T)], axis=1)
    # (d32, n4) partition layout: p = d32*4 + j; sum over the 4 j-partitions.
    # Two 64-col variants: even b -> rows land in cols 0-31, odd b -> cols 32-63.
    sum4 = np.zeros((D_INNER, 2 * 64), f32)
    for i in range(32):
        for j in range(4):
            sum4[4 * i + j, i] = 1.0
            sum4[4 * i + j, 64 + 32 + i] = 1.0
    # diag(D_skip) blocks for the yacc seed matmuls
    dskd = np.zeros((D_INNER, T * D_INNER), f32)
    for l in range(T):
        for blk in (0, 64):
            for q in range(64):
                dskd[blk + q, l * D_INNER + blk + q] = g["D_skip"][l][blk + q]
    A = -np.exp(g["A_log"])
    Acol = np.zeros((D_INNER, T * N_STATE), f32)
    pidx = np.arange(D_INNER)
    for l in range(T):
        for b in range(4):
            for nq in range(4):
                Acol[:, l * N_STATE + 4 * b + nq] = A[l][32 * b + pidx // 4,
                                                         4 * nq + pidx % 4]
    Wout = np.concatenate([g["W_out"][l] for l in range(T)], axis=1)
    # LN stats weights: [I - J/64 | J/64]
    Wstat = np.zeros((D_MODEL, 2 * D_MODEL), f32)
    Wstat[:, 0:D_MODEL] = np.eye(D_MODEL) - 1.0 / D_MODEL
    Wstat[:, D_MODEL:] = 1.0 / D_MODEL

    shared = {
        "Wemb": g["W_emb"].astype(bf),
        "bemb": g["b_emb"].reshape(D_MODEL, 1).astype(f32),
        "peT": np.ascontiguousarray(g["pe"][:L].T).astype(bf),
        "Wstat": Wstat.astype(bf),
        "Win": Win.astype(bf),
        "beta": beta_blob,
        "cdiag": cdiag.astype(bf),
        "bconv": np.ascontiguousarray(g["b_conv"].T).astype(f32),
        "Wxdt": Wxdt.astype(bf),
        "Wdt": Wdt.astype(bf),
        "bdt": np.ascontiguousarray(g["b_dt"].T).astype(f32),
        "WxB": WxB.astype(bf),
        "WxC": WxC.astype(bf),
        "sum4": sum4.astype(bf),
        "dskd": dskd.astype(bf),
        "Acol": Acol.astype(f32),
        "Wout": Wout.astype(bf),
        "Wproj": (g["W_proj"] / L).astype(f32),
        "bproj": g["b_proj"].reshape(OUT_DIM, 1).astype(f32),
    }
    in_maps = []
    for c in range(B):
        m = dict(shared)
        m["xT"] = np.ascontiguousarray(g["x"][c, :L].T).astype(bf)
        in_maps.append(m)
    return in_maps


_CACHE = {}


def kernel(**inputs):
    if "nc" not in _CACHE:
        _CACHE["nc"] = build_nc()
        _CACHE["nc"].finalize()
    nc = _CACHE["nc"]
    in_maps = prep_inputs(inputs)
    from concourse.bass_utils import run_bass_kernel_spmd
    res = run_bass_kernel_spmd(nc, in_maps, core_ids=list(range(N_CORES)))
    out = np.stack([np.asarray(res.results[c]["out"]).reshape(OUT_DIM)
                    for c in range(N_CORES)], axis=0)
    return out.astype(np.float32)

